# revision 8
# baseline (speedup 1.0000x reference)
"""Birman-Schwinger core: K[b] = diag(sqrt|V_b|) @ R_0 @ diag(sqrt|V_b|).

Key identity: with g[b,u] = sqrt(|V[b,u]| + eps) / (1 + u) and d = u - v,

    K[b,u,v] = g[b,u] * g[b,v] * H(d)
    H(d) = 0.5j * exp(2j*d) * sign(d)
         = -0.5*sign(d)*sin(2d)  +  0.5j*sign(d)*cos(2d)

Angle addition splits H into a rank-2 outer product per re/im plane:
with P_u = 0.5 g_u cos2u, Q_u = 0.5 g_u sin2u, X_v = g_v cos2v,
Y_v = g_v sin2v, and j the interleaved re/im f32 column (v = j>>1):

    K_int[u, j] = sign(u-v) * (P_u * A[j] + Q_u * B[j])
    A[2v] = Y_v, A[2v+1] = X_v;  B[2v] = -X_v, B[2v+1] = Y_v

so each (128, 512) output chunk is ONE K=12 bf16 matmul (triple-split
P/Q x triple-split A/B for fp32-grade accuracy) into PSUM, drained by
the Scalar/Vector engines into SBUF store tiles, then DMA'd out.
sign(u-v) is constant +/-1 per chunk except the single diagonal chunk
per row block, which gets a host-built triangular mask during drain.
No Toeplitz table is read from HBM: steady-state traffic is the
(irreducible) 64 MiB of output writes per core.

Sharding: 8 cores; core c handles batch b = c // 2, block parity
h = c % 2: the 16 row blocks u in [256j + 128h, 256j + 128h + 128).
Interleaving blocks this way puts every core's diagonal chunk at local
chunk index j, so one program serves all cores (the mask input data
differs by parity, not the program). Output written as interleaved
re/im f32 pairs = complex64 memory layout.
"""

import numpy as np

B = 4
N = 4096
NCORES = 8
HALF = N // 2            # rows per core
P = 128                  # SBUF partitions
NBLK = HALF // P         # 16 row blocks per core
EPS = 1e-10
KK = 12                  # matmul contraction (triple-split x 2 terms x 2 halves)
CW = 512                 # f32 cols per matmul chunk (1 PSUM bank)
NCHUNK = (2 * N) // CW   # 16 chunks per row block

_PROGRAM_CACHE = {}


def _build_program():
    import concourse.bacc as bacc
    import concourse.mybir as mybir
    from concourse.tile import TileContext

    nc = bacc.Bacc("TRN2", target_bir_lowering=False, debug=False)
    rhs = nc.dram_tensor("t_rhs", [KK, 2 * N], mybir.dt.bfloat16, kind="ExternalInput").ap()
    lhs = nc.dram_tensor("t_lhs", [KK, 2 * HALF], mybir.dt.bfloat16, kind="ExternalInput").ap()
    mask = nc.dram_tensor("t_mask", [P, CW], mybir.dt.float32, kind="ExternalInput").ap()
    out = nc.dram_tensor("t_out", [HALF, 2 * N], mybir.dt.float32, kind="ExternalOutput").ap()
    mult = mybir.AluOpType.mult

    with TileContext(nc) as tc:
        with tc.tile_pool(name="const", bufs=1) as cpool:
            rhs_sb = cpool.tile([KK, 2 * N], mybir.dt.bfloat16)
            lhs_sb = cpool.tile([KK, 2 * HALF], mybir.dt.bfloat16)
            mask_sb = cpool.tile([P, CW], mybir.dt.float32)
            nc.sync.dma_start(out=rhs_sb[:, :], in_=rhs[:, :])
            nc.sync.dma_start(out=lhs_sb[:, :], in_=lhs[:, :])
            # Mask rides the gpsimd SWDGE ring: it isn't needed until the
            # first block's last tile, and this keeps the HWDGE store
            # rings free of input traffic.
            nc.gpsimd.dma_start(out=mask_sb[:, :], in_=mask[:, :])

            with (
                tc.tile_pool(name="psum", bufs=8, space="PSUM") as ppool,
                tc.tile_pool(name="work", bufs=5) as wpool,
            ):
                ci = 0
                # Process block 15 first: its diagonal chunk comes last,
                # keeping the mask load off the critical path. Fine tiles
                # at the very start (first store DMA issues sooner) and
                # at the very end (smaller unoverlapped final drain).
                order = [NBLK - 1] + list(range(NBLK - 1))
                first_widths = [1024, 1024, 2048, 4096]
                last_widths = [4096, 2048, 1024, 1024]
                for bi, j in enumerate(order):
                    if bi == 0:
                        widths = first_widths
                    elif bi == NBLK - 1:
                        widths = last_widths
                    else:
                        widths = [8192]
                    t0 = 0
                    for tw in widths:
                        t = wpool.tile([P, tw], mybir.dt.float32)
                        for cc in range(tw // CW):
                            c = (t0 // CW) + cc
                            pt = ppool.tile([P, CW], mybir.dt.float32)
                            # sign(u-v) is +1 left of the diagonal
                            # chunk, -1 right of it; the negated P/Q
                            # live in the second half of lhs_sb.
                            loff = 0 if c <= j else HALF
                            nc.tensor.matmul(
                                out=pt[:, :],
                                lhsT=lhs_sb[:, loff + P * j : loff + P * (j + 1)],
                                rhs=rhs_sb[:, CW * c : CW * (c + 1)],
                                start=True,
                                stop=True,
                            )
                            dst = t[:, CW * cc : CW * (cc + 1)]
                            if c == j:
                                nc.vector.tensor_tensor(
                                    out=dst, in0=pt[:, :], in1=mask_sb[:, :], op=mult
                                )
                            elif c % 2 == 0:
                                nc.scalar.copy(out=dst, in_=pt[:, :])
                            else:
                                nc.vector.tensor_copy(out=dst, in_=pt[:, :])
                        dma_eng = nc.sync if ci % 2 == 0 else nc.scalar
                        dma_eng.dma_start(
                            out=out[j * P : (j + 1) * P, t0 : t0 + tw], in_=t[:, :]
                        )
                        ci += 1
                        t0 += tw
    nc.compile()
    return nc


def _get_program():
    if "nc" not in _PROGRAM_CACHE:
        _PROGRAM_CACHE["nc"] = _build_program()
    return _PROGRAM_CACHE["nc"]


def _split3(x, bf16):
    """f64 -> three bf16 planes summing to x (~24-bit mantissa)."""
    x0 = x.astype(bf16)
    r1 = x - x0.astype(np.float64)
    x1 = r1.astype(bf16)
    r2 = r1 - x1.astype(np.float64)
    x2 = r2.astype(bf16)
    return x0, x1, x2


def _host_tables(V):
    import ml_dtypes

    bf16 = ml_dtypes.bfloat16
    pos = np.arange(N, dtype=np.float64)
    c2 = np.cos(2.0 * pos)
    s2 = np.sin(2.0 * pos)

    # Triangular diagonal-chunk masks, one per block parity.
    p = np.arange(P, dtype=np.int64)[:, None]
    v = (np.arange(CW, dtype=np.int64) // 2)[None, :]
    masks = [
        np.sign(p - v).astype(np.float32),          # h=0: diag in cols [0,256)
        np.sign(p + P - v).astype(np.float32),      # h=1: diag in cols [256,512)
    ]

    in_maps = []
    for c in range(NCORES):
        b, h = divmod(c, 2)
        g = np.sqrt(np.abs(V[b]).astype(np.float64) + EPS) / (1.0 + pos)
        X = g * c2
        Y = g * s2
        A = np.empty(2 * N)
        A[0::2] = Y
        A[1::2] = X
        Bv = np.empty(2 * N)
        Bv[0::2] = -X
        Bv[1::2] = Y
        Pu = 0.5 * g * c2
        Qu = 0.5 * g * s2
        A0, A1, A2 = _split3(A, bf16)
        B0, B1, B2 = _split3(Bv, bf16)
        P0, P1, P2 = _split3(Pu, bf16)
        Q0, Q1, Q2 = _split3(Qu, bf16)
        rhs12 = np.stack([A0, A1, A0, A2, A1, A0, B0, B1, B0, B2, B1, B0])
        lhs12 = np.stack([P0, P0, P1, P0, P1, P2, Q0, Q0, Q1, Q0, Q1, Q2])
        # This core's rows: u = 256j + 128h + p, j in [0,16), p in [0,128).
        uidx = (256 * np.arange(NBLK)[:, None] + 128 * h + np.arange(P)[None, :]).ravel()
        lhs_pos = lhs12[:, uidx]
        lhs = np.concatenate([lhs_pos, -lhs_pos], axis=1).astype(bf16)
        in_maps.append(
            {
                "t_rhs": np.ascontiguousarray(rhs12),
                "t_lhs": np.ascontiguousarray(lhs),
                "t_mask": masks[h],
            }
        )
    return in_maps


def _run(in_maps, trace=False, **kwargs):
    from concourse import bass_utils

    nc = _get_program()
    return bass_utils.run_bass_kernel_spmd(
        nc, in_maps, core_ids=list(range(NCORES)), trace=trace, **kwargs
    )


def kernel(V):
    V = np.asarray(V, dtype=np.float32)
    assert V.shape == (B, N), V.shape
    in_maps = _host_tables(V)
    res = _run(in_maps, trace=False)
    out = np.empty((B, N, N), dtype=np.complex64)
    for c in range(NCORES):
        b, h = divmod(c, 2)
        plane = np.ascontiguousarray(res.results[c]["t_out"])
        cplane = plane.view(np.complex64)  # (2048, 4096)
        out[b].reshape(NBLK, 2 * P, N)[:, 128 * h : 128 * (h + 1), :] = cplane.reshape(
            NBLK, P, N
        )
    return out


# revision 11
# speedup vs baseline: 1.0055x; 1.0055x over previous
"""Birman-Schwinger core: K[b] = diag(sqrt|V_b|) @ R_0 @ diag(sqrt|V_b|).

Key identity: with g[b,u] = sqrt(|V[b,u]| + eps) / (1 + u) and d = u - v,

    K[b,u,v] = g[b,u] * g[b,v] * H(d)
    H(d) = 0.5j * exp(2j*d) * sign(d)
         = -0.5*sign(d)*sin(2d)  +  0.5j*sign(d)*cos(2d)

Angle addition splits H into a rank-2 outer product per re/im plane:
with P_u = 0.5 g_u cos2u, Q_u = 0.5 g_u sin2u, X_v = g_v cos2v,
Y_v = g_v sin2v, and j the interleaved re/im f32 column (v = j>>1):

    K_int[u, j] = sign(u-v) * (P_u * A[j] + Q_u * B[j])
    A[2v] = Y_v, A[2v+1] = X_v;  B[2v] = -X_v, B[2v+1] = Y_v

so each (128, 512) output chunk is ONE K=12 bf16 matmul (triple-split
P/Q x triple-split A/B for fp32-grade accuracy) into PSUM, drained by
the Scalar/Vector engines into SBUF store tiles, then DMA'd out.
sign(u-v) is constant +/-1 per chunk except the single diagonal chunk
per row block, which gets a host-built triangular mask during drain.
No Toeplitz table is read from HBM: steady-state traffic is the
(irreducible) 64 MiB of output writes per core.

Sharding: 8 cores; core c handles batch b = c // 2, block parity
h = c % 2: the 16 row blocks u in [256j + 128h, 256j + 128h + 128).
Interleaving blocks this way puts every core's diagonal chunk at local
chunk index j, so one program serves all cores (the mask input data
differs by parity, not the program). Output written as interleaved
re/im f32 pairs = complex64 memory layout.
"""

import numpy as np

B = 4
N = 4096
NCORES = 8
HALF = N // 2            # rows per core
P = 128                  # SBUF partitions
NBLK = HALF // P         # 16 row blocks per core
EPS = 1e-10
KK = 12                  # matmul contraction (triple-split x 2 terms x 2 halves)
CW = 512                 # f32 cols per matmul chunk (1 PSUM bank)
NCHUNK = (2 * N) // CW   # 16 chunks per row block

_PROGRAM_CACHE = {}


def _build_program():
    import concourse.bacc as bacc
    import concourse.mybir as mybir
    from concourse.tile import TileContext

    nc = bacc.Bacc("TRN2", target_bir_lowering=False, debug=False)
    rhs = nc.dram_tensor("t_rhs", [KK, 2 * N], mybir.dt.bfloat16, kind="ExternalInput").ap()
    lhs = nc.dram_tensor("t_lhs", [KK, 2 * HALF], mybir.dt.bfloat16, kind="ExternalInput").ap()
    mask = nc.dram_tensor("t_mask", [P, CW], mybir.dt.float32, kind="ExternalInput").ap()
    out = nc.dram_tensor("t_out", [HALF, 2 * N], mybir.dt.float32, kind="ExternalOutput").ap()
    mult = mybir.AluOpType.mult

    with TileContext(nc) as tc:
        with tc.tile_pool(name="const", bufs=1) as cpool:
            rhs_sb = cpool.tile([KK, 2 * N], mybir.dt.bfloat16)
            lhs_sb = cpool.tile([KK, 2 * HALF], mybir.dt.bfloat16)
            mask_sb = cpool.tile([P, CW], mybir.dt.float32)
            nc.sync.dma_start(out=rhs_sb[:, :], in_=rhs[:, :])
            nc.sync.dma_start(out=lhs_sb[:, :], in_=lhs[:, :])
            # Mask rides the gpsimd SWDGE ring: it isn't needed until the
            # first block's last tile, and this keeps the HWDGE store
            # rings free of input traffic.
            nc.gpsimd.dma_start(out=mask_sb[:, :], in_=mask[:, :])

            with (
                tc.tile_pool(name="psum", bufs=8, space="PSUM") as ppool,
                tc.tile_pool(name="work", bufs=9) as wpool,
            ):
                ci = 0
                # Process block 15 first: its diagonal chunk comes last,
                # keeping the mask load off the critical path. Fine tiles
                # at the very start (first store DMA issues sooner) and
                # at the very end (smaller unoverlapped final drain).
                order = [NBLK - 1] + list(range(NBLK - 1))
                first_widths = [512, 512, 1024, 2048, 4096]
                last_widths = [4096, 2048, 1024, 512, 512]
                for bi, j in enumerate(order):
                    if bi == 0:
                        widths = first_widths
                    elif bi == NBLK - 1:
                        widths = last_widths
                    else:
                        widths = [4096, 4096]
                    t0 = 0
                    for tw in widths:
                        t = wpool.tile([P, tw], mybir.dt.float32)
                        for cc in range(tw // CW):
                            c = (t0 // CW) + cc
                            pt = ppool.tile([P, CW], mybir.dt.float32)
                            # sign(u-v) is +1 left of the diagonal
                            # chunk, -1 right of it; the negated P/Q
                            # live in the second half of lhs_sb.
                            loff = 0 if c <= j else HALF
                            nc.tensor.matmul(
                                out=pt[:, :],
                                lhsT=lhs_sb[:, loff + P * j : loff + P * (j + 1)],
                                rhs=rhs_sb[:, CW * c : CW * (c + 1)],
                                start=True,
                                stop=True,
                            )
                            dst = t[:, CW * cc : CW * (cc + 1)]
                            if c == j:
                                nc.vector.tensor_tensor(
                                    out=dst, in0=pt[:, :], in1=mask_sb[:, :], op=mult
                                )
                            elif c % 2 == 0:
                                nc.scalar.copy(out=dst, in_=pt[:, :])
                            else:
                                nc.vector.tensor_copy(out=dst, in_=pt[:, :])
                        dma_eng = nc.sync if ci % 2 == 0 else nc.scalar
                        dma_eng.dma_start(
                            out=out[j * P : (j + 1) * P, t0 : t0 + tw], in_=t[:, :]
                        )
                        ci += 1
                        t0 += tw
    nc.compile()
    return nc


def _get_program():
    if "nc" not in _PROGRAM_CACHE:
        _PROGRAM_CACHE["nc"] = _build_program()
    return _PROGRAM_CACHE["nc"]


def _split3(x, bf16):
    """f64 -> three bf16 planes summing to x (~24-bit mantissa)."""
    x0 = x.astype(bf16)
    r1 = x - x0.astype(np.float64)
    x1 = r1.astype(bf16)
    r2 = r1 - x1.astype(np.float64)
    x2 = r2.astype(bf16)
    return x0, x1, x2


def _host_tables(V):
    import ml_dtypes

    bf16 = ml_dtypes.bfloat16
    pos = np.arange(N, dtype=np.float64)
    c2 = np.cos(2.0 * pos)
    s2 = np.sin(2.0 * pos)

    # Triangular diagonal-chunk masks, one per block parity.
    p = np.arange(P, dtype=np.int64)[:, None]
    v = (np.arange(CW, dtype=np.int64) // 2)[None, :]
    masks = [
        np.sign(p - v).astype(np.float32),          # h=0: diag in cols [0,256)
        np.sign(p + P - v).astype(np.float32),      # h=1: diag in cols [256,512)
    ]

    in_maps = []
    for c in range(NCORES):
        b, h = divmod(c, 2)
        g = np.sqrt(np.abs(V[b]).astype(np.float64) + EPS) / (1.0 + pos)
        X = g * c2
        Y = g * s2
        A = np.empty(2 * N)
        A[0::2] = Y
        A[1::2] = X
        Bv = np.empty(2 * N)
        Bv[0::2] = -X
        Bv[1::2] = Y
        Pu = 0.5 * g * c2
        Qu = 0.5 * g * s2
        A0, A1, A2 = _split3(A, bf16)
        B0, B1, B2 = _split3(Bv, bf16)
        P0, P1, P2 = _split3(Pu, bf16)
        Q0, Q1, Q2 = _split3(Qu, bf16)
        rhs12 = np.stack([A0, A1, A0, A2, A1, A0, B0, B1, B0, B2, B1, B0])
        lhs12 = np.stack([P0, P0, P1, P0, P1, P2, Q0, Q0, Q1, Q0, Q1, Q2])
        # This core's rows: u = 256j + 128h + p, j in [0,16), p in [0,128).
        uidx = (256 * np.arange(NBLK)[:, None] + 128 * h + np.arange(P)[None, :]).ravel()
        lhs_pos = lhs12[:, uidx]
        lhs = np.concatenate([lhs_pos, -lhs_pos], axis=1).astype(bf16)
        in_maps.append(
            {
                "t_rhs": np.ascontiguousarray(rhs12),
                "t_lhs": np.ascontiguousarray(lhs),
                "t_mask": masks[h],
            }
        )
    return in_maps


def _run(in_maps, trace=False, **kwargs):
    from concourse import bass_utils

    nc = _get_program()
    return bass_utils.run_bass_kernel_spmd(
        nc, in_maps, core_ids=list(range(NCORES)), trace=trace, **kwargs
    )


def kernel(V):
    V = np.asarray(V, dtype=np.float32)
    assert V.shape == (B, N), V.shape
    in_maps = _host_tables(V)
    res = _run(in_maps, trace=False)
    out = np.empty((B, N, N), dtype=np.complex64)
    for c in range(NCORES):
        b, h = divmod(c, 2)
        plane = np.ascontiguousarray(res.results[c]["t_out"])
        cplane = plane.view(np.complex64)  # (2048, 4096)
        out[b].reshape(NBLK, 2 * P, N)[:, 128 * h : 128 * (h + 1), :] = cplane.reshape(
            NBLK, P, N
        )
    return out
